# revision 7
# baseline (speedup 1.0000x reference)
"""Trainium2 Bass kernel for nn_CoreBlock (circulant attention + 2-layer FFN).

Contract: kernel(**inputs) takes FULL unsharded inputs (as produced by
setup_inputs) and returns the FULL [16, 1024, 768] f32 output.

v2 strategy (data-parallel over batch, 8 cores x 2 batches):
  - All on-chip transposes done by the DMA-transpose XBAR (bf16), not the PE.
  - x is kept in SBUF only as bf16 (XB); phase B adds the circulant result
    in place so XB becomes x1.
  - Circulant matmul: per (head, batch) 15 fat matmuls (F up to 512) instead
    of 64 thin F=64 ones: for each diagonal m the stationary Toeplitz tile
    T[h,m] streams V slots m..7 and 0..m-1 (wrap split), accumulating the
    full [128, 8*64] PSUM bank.
  - Phase C is software-pipelined: layer 0 in halves, layer 1 + log_cosh
    tail in quarters, so the scalar-engine tail (exp/ln) overlaps the PE.
  - Scalar activation-table locality: phase A uses Copy+Sqrt only; each
    phase-C group costs Sqrt/Silu/(Exp+Ln) table loads.
"""

import math
import numpy as np
import ml_dtypes

import concourse.bass as bass
import concourse.tile as tile
from concourse import bacc, mybir
from concourse.bass_utils import run_bass_kernel_spmd

BF16 = ml_dtypes.bfloat16

B, N, D = 16, 1024, 768
H, HS, L = 12, 64, 2
EPS = 1e-6
NCORES = 8
BPC = B // NCORES          # batches per core (2)
NJ = N // 128              # token chunks per batch (8)
NT = BPC * NJ              # token chunks per core (16)
NP = NT // 2               # token pairs per core (8)
DC = D // 128              # feature chunks (6)

F32 = mybir.dt.float32
BF = mybir.dt.bfloat16
Alu = mybir.AluOpType
Act = mybir.ActivationFunctionType

TRACE = False              # test harness sets this for profiling runs
TRACE_KW = {}

_cache = {}


def _build(cv_nonzero, bf_nonzero, lnf_uniform):
    nc = bacc.Bacc("TRN2", target_bir_lowering=False, debug=False)

    xs = nc.dram_tensor("xs", (BPC, N, D), F32, kind="ExternalInput").ap()
    wv = nc.dram_tensor("wv", (D, D), BF, kind="ExternalInput").ap()
    wf = nc.dram_tensor("wf", (L, D, D), BF, kind="ExternalInput").ap()
    tb_d = nc.dram_tensor("tbank", (H, 128, NJ * 128), BF, kind="ExternalInput").ap()
    cv_d = nc.dram_tensor("cv", (1, D), F32, kind="ExternalInput").ap()
    bf_d = nc.dram_tensor("bfb", (L, D), F32, kind="ExternalInput").ap()
    lnfs_d = nc.dram_tensor("lnfs", (L, D), F32, kind="ExternalInput").ap()
    lnfb_d = nc.dram_tensor("lnfb", (L, D), F32, kind="ExternalInput").ap()
    out_d = nc.dram_tensor("out", (BPC, N, D), F32, kind="ExternalOutput").ap()

    with tile.TileContext(nc) as tc:
        _emit(nc, tc, xs, wv, wf, tb_d, cv_d, bf_d, lnfs_d, lnfb_d, out_d,
              cv_nonzero, bf_nonzero, lnf_uniform)
    nc.compile()
    return nc


def _emit(nc, tc, xs, wv, wf, tb_d, cv_d, bf_d, lnfs_d, lnfb_d, out_d,
          cv_nonzero, bf_nonzero, lnf_uniform):
    from contextlib import ExitStack
    ctx = ExitStack()
    inv_d = 1.0 / D
    ln2 = math.log(2.0)
    with ctx:
        consts = ctx.enter_context(tc.tile_pool(name="consts", bufs=1))
        xbp = ctx.enter_context(tc.tile_pool(name="xbp", bufs=1))
        vpool = ctx.enter_context(tc.tile_pool(name="vpool", bufs=1))
        statp = ctx.enter_context(tc.tile_pool(name="statp", bufs=1))
        xtmp = ctx.enter_context(tc.tile_pool(name="xtmp", bufs=2))
        u2p = ctx.enter_context(tc.tile_pool(name="u2p", bufs=2))
        dtp = ctx.enter_context(tc.tile_pool(name="dtp", bufs=3))
        scrp = ctx.enter_context(tc.tile_pool(name="scrp", bufs=2))
        y0p = ctx.enter_context(tc.tile_pool(name="y0p", bufs=8))
        y1p = ctx.enter_context(tc.tile_pool(name="y1p", bufs=4))
        wp = ctx.enter_context(tc.tile_pool(name="wp", bufs=3))
        ebp = ctx.enter_context(tc.tile_pool(name="ebp", bufs=3))
        otp = ctx.enter_context(tc.tile_pool(name="otp", bufs=2))
        tbp = ctx.enter_context(tc.tile_pool(name="tbp", bufs=2))
        ps_mm = ctx.enter_context(tc.tile_pool(name="ps_mm", bufs=2, space="PSUM"))
        ps_b = ctx.enter_context(tc.tile_pool(name="ps_b", bufs=2, space="PSUM"))

        # ---- constants ----
        wv_s = consts.tile([128, DC, D], BF, tag="wv")
        wf_s = consts.tile([128, L, DC, D], BF, tag="wf")
        epst = consts.tile([128, 1], F32, tag="eps")
        nc.vector.memset(epst[:], EPS)
        zerot = consts.tile([128, 1], F32, tag="zero")
        nc.vector.memset(zerot[:], 0.0)
        onet = consts.tile([128, 1], F32, tag="one")
        nc.vector.memset(onet[:], 1.0)
        bft = consts.tile([128, L, D], F32, tag="bft")
        cvt = None
        if cv_nonzero:
            cvt = consts.tile([128, D], F32, tag="cv")
        lnfst = [None] * L
        lnfbt = [None] * L
        for l in range(L):
            if lnf_uniform[l] is None:
                lnfst[l] = consts.tile([128, D], F32, tag=f"lnfs{l}")
                lnfbt[l] = consts.tile([128, D], F32, tag=f"lnfb{l}")

        def load_late_consts():
            for l in range(L):
                nc.scalar.dma_start(bft[:, l, :],
                                    bf_d[l:l + 1].to_broadcast((128, D)))
                if lnf_uniform[l] is None:
                    nc.sync.dma_start(lnfst[l][:],
                                      lnfs_d[l:l + 1].to_broadcast((128, D)))
                    nc.sync.dma_start(lnfbt[l][:],
                                      lnfb_d[l:l + 1].to_broadcast((128, D)))

        # ---- resident tensors ----
        XB = xbp.tile([128, BPC, NJ, D], BF, tag="XB")        # bf16 x, then x1
        V = vpool.tile([128, H, NJ, BPC, HS], BF, tag="V")    # per-head values

        sumsA = statp.tile([128, NT], F32, tag="sumsA")
        ssqA = statp.tile([128, NT], F32, tag="ssqA")
        muA = statp.tile([128, NT], F32, tag="muA")
        m2A = statp.tile([128, NT], F32, tag="m2A")
        varA = statp.tile([128, NT], F32, tag="varA")
        sdA = statp.tile([128, NT], F32, tag="sdA")
        rsA = statp.tile([128, NT], F32, tag="rsA")

        def vproj_mm(udt, half, pv):
            for c in range(DC):
                nc.tensor.matmul(pv[:, 0:512], udt[:, DC * half + c, :],
                                 wv_s[:, c, 0:512],
                                 start=(c == 0), stop=(c == DC - 1))
                nc.tensor.matmul(pv[:, 512:D], udt[:, DC * half + c, :],
                                 wv_s[:, c, 512:D],
                                 start=(c == 0), stop=(c == DC - 1))

        def phase_b(b):
            for h2 in range(0, H, 2):
                tb2 = tbp.tile([128, 2, NJ, 128], BF, tag="tb")
                nc.scalar.dma_start(
                    tb2[:], tb_d[h2:h2 + 2].rearrange("h p (m f) -> p h m f", m=NJ))
                for hh in range(2):
                    h = h2 + hh
                    pc = ps_b.tile([128, NJ, HS], F32, tag="pb")
                    for m in range(NJ):
                        rem = NJ - m
                        nc.tensor.matmul(pc[:, 0:rem, :], tb2[:, hh, m, :],
                                         V[:, h, m:NJ, b, :],
                                         start=(m == 0), stop=(m == NJ - 1),
                                         skip_group_check=True)
                        if m > 0:
                            nc.tensor.matmul(pc[:, rem:NJ, :], tb2[:, hh, m, :],
                                             V[:, h, 0:m, b, :],
                                             start=False, stop=(m == NJ - 1),
                                             skip_group_check=True)
                    xap = XB[:, b, :, h * HS:(h + 1) * HS]    # [128, NJ, HS]
                    nc.vector.tensor_tensor(xap, xap, pc[:], op=Alu.add)

        # ================= phase A: load, stats, v-projection =================
        def phase_a(b):
            for jp in range(NJ // 2):
                pi = b * (NJ // 2) + jp
                xt = xtmp.tile([128, 2, D], F32, tag="xt")
                nc.scalar.dma_start(
                    xt[:], xs[b, jp * 256:(jp + 1) * 256, :]
                    .rearrange("(c p) f -> p c f", p=128))
                for ci in range(2):
                    t = 2 * pi + ci
                    jc = 2 * jp + ci
                    xbt = XB[:, b, jc, :]
                    nc.scalar.activation(xbt, xt[:, ci, :], Act.Copy,
                                         accum_out=sumsA[:, t:t + 1])
                    scr = scrp.tile([128, D], BF, tag="scr")
                    nc.vector.scalar_tensor_tensor(
                        scr[:], xbt, 0.0, xbt, op0=Alu.add, op1=Alu.mult,
                        accum_out=ssqA[:, t:t + 1])
                # per-pair stats -> first matmul starts early
                s2 = slice(2 * pi, 2 * pi + 2)
                nc.vector.tensor_scalar(muA[:, s2], sumsA[:, s2], inv_d, None,
                                        op0=Alu.mult)
                nc.vector.tensor_scalar(m2A[:, s2], ssqA[:, s2], inv_d, None,
                                        op0=Alu.mult)
                nc.vector.scalar_tensor_tensor(varA[:, s2], muA[:, s2], -1.0,
                                               muA[:, s2], op0=Alu.mult,
                                               op1=Alu.mult)
                nc.vector.tensor_tensor(varA[:, s2], m2A[:, s2], varA[:, s2],
                                        op=Alu.add)
                nc.scalar.activation(sdA[:, s2], varA[:, s2], Act.Sqrt,
                                     bias=epst[:])
                nc.vector.reciprocal(rsA[:, s2], sdA[:, s2])
                u2 = u2p.tile([128, 2, D], BF, tag="u2")
                for ci in range(2):
                    t = 2 * pi + ci
                    jc = 2 * jp + ci
                    nc.vector.tensor_scalar(u2[:, ci, :], XB[:, b, jc, :],
                                            muA[:, t:t + 1], rsA[:, t:t + 1],
                                            op0=Alu.subtract, op1=Alu.mult)
                udt = dtp.tile([128, 2 * DC, 128], BF, tag="udt")
                nc.sync.dma_start(udt[:], u2[:], transpose=True)
                for ci in range(2):
                    jc = 2 * jp + ci
                    pv = ps_mm.tile([128, D], F32, tag="mm")
                    vproj_mm(udt, ci, pv)
                    vdst = V[:, :, jc, b, :]                  # [128, H, HS]
                    pv3 = pv[:].rearrange("p (h k) -> p h k", h=H)
                    if cv_nonzero:
                        cv3 = cvt[:].rearrange("p (h k) -> p h k", h=H)
                        nc.vector.tensor_tensor(vdst, pv3, cv3, op=Alu.add)
                    else:
                        nc.scalar.activation(vdst, pv3, Act.Copy)

        # ================= phase C stages =================
        def ffn_mm(zdt, half, pf, l):
            for c in range(DC):
                nc.tensor.matmul(pf[:, 0:512], zdt[:, DC * half + c, :],
                                 wf_s[:, l, c, 0:512],
                                 start=(c == 0), stop=(c == DC - 1))
                nc.tensor.matmul(pf[:, 512:D], zdt[:, DC * half + c, :],
                                 wf_s[:, l, c, 512:D],
                                 start=(c == 0), stop=(c == DC - 1))

        def ln_group(l, s, sums, ssq, rsF, biasF):
            mu = statp.tile([128, NT], F32, tag=f"muC{l}")
            m2 = statp.tile([128, NT], F32, tag=f"m2C{l}")
            var = statp.tile([128, NT], F32, tag=f"varC{l}")
            sd = statp.tile([128, NT], F32, tag=f"sdC{l}")
            nc.vector.tensor_scalar(mu[:, s], sums[:, s], inv_d, None, op0=Alu.mult)
            nc.vector.tensor_scalar(m2[:, s], ssq[:, s], inv_d, None, op0=Alu.mult)
            nc.vector.scalar_tensor_tensor(var[:, s], mu[:, s], -1.0, mu[:, s],
                                           op0=Alu.mult, op1=Alu.mult)
            nc.vector.tensor_tensor(var[:, s], m2[:, s], var[:, s], op=Alu.add)
            nc.scalar.activation(sd[:, s], var[:, s], Act.Sqrt, bias=epst[:])
            nc.vector.reciprocal(rsF[:, s], sd[:, s])
            if lnf_uniform[l] is not None:
                cs, cb = lnf_uniform[l]
                if cs != 1.0:
                    nc.vector.tensor_scalar(rsF[:, s], rsF[:, s], float(cs), None,
                                            op0=Alu.mult)
                nc.vector.scalar_tensor_tensor(biasF[:, s], mu[:, s], -1.0,
                                               rsF[:, s], op0=Alu.mult,
                                               op1=Alu.mult)
                if cb != 0.0:
                    nc.vector.tensor_scalar(biasF[:, s], biasF[:, s], float(cb),
                                            None, op0=Alu.add)
            else:
                nc.vector.tensor_copy(biasF[:, s], mu[:, s])

        def act_chunk(l, y2, ci, t, rsF, biasF):
            ysl = y2[:, ci, :]
            if lnf_uniform[l] is not None:
                nc.scalar.activation(ysl, ysl, Act.Silu,
                                     bias=biasF[:, t:t + 1], scale=rsF[:, t:t + 1])
            else:
                tmp = scrp.tile([128, D], BF, tag="scr")
                nc.vector.tensor_scalar(tmp[:], ysl, biasF[:, t:t + 1],
                                        rsF[:, t:t + 1],
                                        op0=Alu.subtract, op1=Alu.mult)
                nc.vector.tensor_tensor(tmp[:], tmp[:], lnfst[l][:], op=Alu.mult)
                nc.vector.tensor_tensor(tmp[:], tmp[:], lnfbt[l][:], op=Alu.add)
                nc.scalar.activation(ysl, tmp[:], Act.Silu, bias=zerot[:])

        sums0 = statp.tile([128, NT], F32, tag="sums0")
        ssq0 = statp.tile([128, NT], F32, tag="ssq0")
        rs0 = statp.tile([128, NT], F32, tag="rs0")
        bias0 = statp.tile([128, NT], F32, tag="bias0")
        sums1 = statp.tile([128, NT], F32, tag="sums1")
        ssq1 = statp.tile([128, NT], F32, tag="ssq1")
        rs1 = statp.tile([128, NT], F32, tag="rs1")
        bias1 = statp.tile([128, NT], F32, tag="bias1")
        y0s = [None] * NP

        def c0_mm(b):
            """Layer-0 matmuls + sums/ssq for batch b (8 chunks)."""
            for jp in range(NJ // 2):
                pi = b * (NJ // 2) + jp
                zdt = dtp.tile([128, 2 * DC, 128], BF, tag="udt")
                nc.sync.dma_start(zdt[:], XB[:, b, 2 * jp:2 * jp + 2, :],
                                  transpose=True)
                y0 = y0p.tile([128, 2, D], BF, tag="y0")
                y0s[pi] = y0
                for ci in range(2):
                    t = 2 * pi + ci
                    pf = ps_mm.tile([128, D], F32, tag="mm")
                    ffn_mm(zdt, ci, pf, 0)
                    if bf_nonzero[0]:
                        nc.vector.scalar_tensor_tensor(
                            y0[:, ci, :], pf[:], 0.0, bft[:, 0, :],
                            op0=Alu.add, op1=Alu.add, accum_out=sums0[:, t:t + 1])
                    else:
                        nc.scalar.activation(y0[:, ci, :], pf[:], Act.Copy,
                                             accum_out=sums0[:, t:t + 1])
                    scr = scrp.tile([128, D], BF, tag="scr")
                    nc.vector.scalar_tensor_tensor(
                        scr[:], y0[:, ci, :], 0.0, y0[:, ci, :],
                        op0=Alu.add, op1=Alu.mult, accum_out=ssq0[:, t:t + 1])

        def c0_post(b):
            s = slice(8 * b, 8 * b + 8)
            ln_group(0, s, sums0, ssq0, rs0, bias0)
            for jp in range(NJ // 2):
                pi = b * (NJ // 2) + jp
                for ci in range(2):
                    act_chunk(0, y0s[pi], ci, 2 * pi + ci, rs0, bias0)

        def c1_tail(b, groups):
            """Layer 1 + log_cosh tail for batch b, in `groups` stat groups."""
            npb = NJ // 2                    # pairs per batch
            gp = npb // groups               # pairs per group
            for g in range(groups):
                y1s = {}
                for jj in range(gp):
                    jp = g * gp + jj
                    pi = b * npb + jp
                    zdt = dtp.tile([128, 2 * DC, 128], BF, tag="udt")
                    nc.sync.dma_start(zdt[:], y0s[pi][:], transpose=True)
                    y1 = y1p.tile([128, 2, D], BF, tag="y1")
                    y1s[pi] = y1
                    for ci in range(2):
                        t = 2 * pi + ci
                        pf = ps_mm.tile([128, D], F32, tag="mm")
                        ffn_mm(zdt, ci, pf, 1)
                        nc.vector.scalar_tensor_tensor(
                            y1[:, ci, :], pf[:], 0.0, bft[:, 1, :],
                            op0=Alu.add, op1=Alu.add, accum_out=sums1[:, t:t + 1])
                        scr = scrp.tile([128, D], BF, tag="scr")
                        nc.vector.scalar_tensor_tensor(
                            scr[:], y1[:, ci, :], 0.0, y1[:, ci, :],
                            op0=Alu.add, op1=Alu.mult, accum_out=ssq1[:, t:t + 1])
                s = slice(2 * (b * npb + g * gp), 2 * (b * npb + (g + 1) * gp))
                ln_group(1, s, sums1, ssq1, rs1, bias1)
                for jj in range(gp):
                    pi = b * npb + g * gp + jj
                    for ci in range(2):
                        act_chunk(1, y1s[pi], ci, 2 * pi + ci, rs1, bias1)
                # tail: log_cosh(w) = |w| + log1p(exp(-2|w|)) - log2
                tails = []
                for jj in range(gp):
                    jp = g * gp + jj
                    pi = b * npb + jp
                    w2 = wp.tile([128, 2, D], BF, tag="w2")
                    for ci in range(2):
                        jc = 2 * jp + ci
                        nc.vector.tensor_tensor(w2[:, ci, :], XB[:, b, jc, :],
                                                y1s[pi][:, ci, :], op=Alu.add)
                    nc.vector.scalar_tensor_tensor(
                        w2[:], w2[:], -1.0, w2[:], op0=Alu.mult, op1=Alu.max)
                    tails.append((jp, w2))
                ebs = []
                for jp, w2 in tails:
                    eb2 = ebp.tile([128, 2, D], BF, tag="eb2")
                    nc.scalar.activation(eb2[:], w2[:], Act.Exp,
                                         bias=zerot[:], scale=-2.0)
                    ebs.append(eb2)
                for (jp, w2), eb2 in zip(tails, ebs):
                    nc.scalar.activation(eb2[:], eb2[:], Act.Ln,
                                         bias=onet[:], scale=1.0)
                for (jp, w2), eb2 in zip(tails, ebs):
                    ot2 = otp.tile([128, 2, D], F32, tag="ot2")
                    nc.vector.scalar_tensor_tensor(
                        ot2[:], w2[:], -ln2, eb2[:], op0=Alu.add, op1=Alu.add)
                    nc.sync.dma_start(
                        out_d[b, jp * 256:(jp + 1) * 256, :]
                        .rearrange("(c p) f -> p c f", p=128), ot2[:])

        # ================= emission schedule =================
        nc.sync.dma_start(wv_s[:], wv.rearrange("(c p) f -> p c f", p=128))
        if cv_nonzero:
            nc.scalar.dma_start(cvt[:], cv_d.to_broadcast((128, D)))
        phase_a(0)
        phase_b(0)
        load_late_consts()
        phase_a(1)
        nc.scalar.dma_start(wf_s[:], wf.rearrange("l (c p) f -> p l c f", p=128))
        c0_mm(0)
        c0_post(0)
        phase_b(1)
        c0_mm(1)
        c0_post(1)
        c1_tail(0, 1)
        c1_tail(1, 2)

def _prep(inputs):
    x = np.asarray(inputs["x"], np.float32)
    ln1_s = np.asarray(inputs["ln1_scale"], np.float32)
    ln1_b = np.asarray(inputs["ln1_bias"], np.float32)
    Wv = np.asarray(inputs["Wv"], np.float32)
    alpha = np.asarray(inputs["alpha"], np.float32)
    Wf = np.asarray(inputs["Wf"], np.float32)
    bfv = np.asarray(inputs["bf"], np.float32)
    lnf_s = np.asarray(inputs["lnf_scale"], np.float32)
    lnf_b = np.asarray(inputs["lnf_bias"], np.float32)

    Wv_flat = Wv.transpose(1, 0, 2).reshape(D, H * HS)
    Wvp = (ln1_s[:, None] * Wv_flat).astype(BF16)
    cv = (ln1_b @ Wv_flat).astype(np.float32)

    ar = alpha[:, (-np.arange(N)) % N]
    ar2 = np.concatenate([ar, ar], axis=1)
    m_ = np.arange(NJ)[:, None, None]
    p_ = np.arange(128)[None, :, None]
    f_ = np.arange(128)[None, None, :]
    T = ar2[:, N + 128 * m_ + p_ - f_]                  # [H, NJ, 128, 128]
    tbank = np.ascontiguousarray(
        T.transpose(0, 2, 1, 3).reshape(H, 128, NJ * 128)).astype(BF16)

    cv_nonzero = bool(np.any(cv))
    bf_nonzero = tuple(bool(np.any(bfv[l])) for l in range(L))
    lnf_uniform = []
    for l in range(L):
        s, bb = lnf_s[l], lnf_b[l]
        if np.all(s == s[0]) and np.all(bb == bb[0]):
            lnf_uniform.append((float(s[0]), float(bb[0])))
        else:
            lnf_uniform.append(None)
    key = (cv_nonzero, bf_nonzero, tuple(lnf_uniform))

    common = {
        "wv": np.ascontiguousarray(Wvp),
        "wf": Wf.astype(BF16),
        "tbank": tbank,
        "cv": cv.reshape(1, D),
        "bfb": bfv,
        "lnfs": lnf_s,
        "lnfb": lnf_b,
    }
    return x, key, common, (cv_nonzero, bf_nonzero, lnf_uniform)


def kernel(**inputs):
    x, key, common, flags = _prep(inputs)
    if key not in _cache:
        _cache[key] = _build(*flags)
    nc = _cache[key]
    in_maps = []
    for i in range(NCORES):
        m = dict(common)
        m["xs"] = np.ascontiguousarray(x[i * BPC:(i + 1) * BPC])
        in_maps.append(m)
    res = run_bass_kernel_spmd(nc, in_maps, core_ids=list(range(NCORES)),
                               trace=TRACE, **TRACE_KW)
    kernel.last_result = res
    out = np.empty((B, N, D), np.float32)
    for i in range(NCORES):
        out[i * BPC:(i + 1) * BPC] = res.results[i]["out"]
    return out
